# revision 23
# baseline (speedup 1.0000x reference)
"""Multi-head attention (B=4, S=2048, D=1024, H=16) on 8 trn2 NeuronCores.

Sharding: tensor-parallel over heads — core c owns heads (2c, 2c+1).
Per core:
  1. Q^T/K^T projections into [head_dim-stacked, tokens] layout (bf16),
     V^T projection + PE-transpose into V-natural [tokens, head_dim] with a
     fused ones-column (for softmax row sums).
  2. Attention per (batch, q-window): scores^T = K^T.T @ Q^T (2-head
     row-packed matmuls), exp on ScalarE (PSUM -> SBUF bf16), PV with
     lhsT=[V|1] so PSUM row 64 accumulates the softmax denominator.
     Normalize via reciprocal + DMA partition-broadcast.
  3. AllToAll (heads-sharded -> token-sharded), then the output projection
     for this core's 1024-token slice.

Host side folds the 1/sqrt(head_dim) scale into w_q, pre-transposes and
pre-tiles all operands to bf16, and assembles the [4,2048,1024] fp32 output.

Biases are applied exactly on the host: b_v and b_o contribute
(b_v @ w_o.T + b_o) to every token (softmax rows sum to 1). b_q/b_k cannot
be folded; setup_inputs() generates them as zeros — a numpy fallback guards
the (never-exercised) nonzero case, as well as non-trivial masks.
"""

import numpy as np
import ml_dtypes

import concourse.bass as bass
import concourse.tile as tile
from concourse import mybir
from concourse.bass_utils import run_bass_kernel_spmd
from concourse.masks import make_identity

NCORES = 8
B, S, D, H = 4, 2048, 1024, 16
HD = D // H            # 64
P = 128
T = B * S              # 8192 tokens
TOK_PER_CORE = T // NCORES   # 1024
NCH = D // P           # 8 contraction chunks
NT2 = T // 1024        # 8 token tiles of 1024 for projections
NKT = S // P           # 16 key tiles per batch
NQW = S // 512         # 4 q-windows of 512 per batch
VROW = 2 * (HD + 1)    # 130 cols per k-tile in v_all ([V_h0|1|V_h1|1])

BF16 = mybir.dt.bfloat16
F32 = mybir.dt.float32
bf16 = ml_dtypes.bfloat16

_CACHED_NC = None


def split_multi_waits(nc):
    """This walrus build supports one sync-wait per instruction; hoist extras
    onto same-engine NoOps inserted immediately before."""
    for f in nc.m.functions:
        for blk in f.blocks:
            insts = blk.instructions
            i = 0
            while i < len(insts):
                inst = insts[i]
                si = getattr(inst, "sync_info", None)
                if si is not None and si.on_wait and len(si.on_wait) > 1:
                    waits = list(si.on_wait)
                    for j, w in enumerate(waits[:-1]):
                        nop = mybir.InstNoOp(name=f"I-ws-{inst.name}-{j}",
                                             ins=[], outs=[])
                        nop.engine = inst.engine
                        nop.sync_info = mybir.SyncInfo(on_wait=[w], on_update=[])
                        insts.insert(i, nop)
                        i += 1
                    inst.sync_info = mybir.SyncInfo(on_wait=[waits[-1]],
                                                    on_update=si.on_update)
                i += 1


def build(split=True):
    global _CACHED_NC
    if split and _CACHED_NC is not None:
        return _CACHED_NC
    from contextlib import ExitStack

    nc = bass.Bass(num_devices=NCORES, target_bir_lowering=False, debug=False)

    # Inputs (per core). x* are the full activations, tiled on host to
    # [toktile, chunk, 128, 1024] so each projection input is one DMA.
    xq_d = nc.dram_tensor("xq", [NT2, NCH, P, 1024], BF16, kind="ExternalInput")
    xk_d = nc.dram_tensor("xk", [NT2, NCH, P, 1024], BF16, kind="ExternalInput")
    xv_d = nc.dram_tensor("xv", [NT2, NCH, P, 1024], BF16, kind="ExternalInput")
    wq_d = nc.dram_tensor("wq", [NCH, P, P], BF16, kind="ExternalInput")
    wk_d = nc.dram_tensor("wk", [NCH, P, P], BF16, kind="ExternalInput")
    wv_d = nc.dram_tensor("wv", [NCH, P, P], BF16, kind="ExternalInput")
    wo_d = nc.dram_tensor("wo", [NCH, P, 1024], BF16, kind="ExternalInput")
    out_d = nc.dram_tensor("out", [TOK_PER_CORE, D], F32, kind="ExternalOutput")

    # Internal DRAM: rowsum/reciprocal bounces + two half-token AllToAll stages.
    rs_d = nc.dram_tensor("rs_d", [B * NQW, 1024], F32)
    rcp_d = nc.dram_tensor("rcp_d", [B * NQW, 1024], F32)
    a2a_in = [nc.dram_tensor(f"a2a_in{h}", [NCORES, P, 512], BF16) for h in (0, 1)]
    a2a_out = [nc.dram_tensor(f"a2a_out{h}", [NCORES, P, 512], BF16) for h in (0, 1)]

    with tile.TileContext(nc, pool_alloc_mode="queue") as tc:
        with ExitStack() as ctx:
            const = ctx.enter_context(tc.tile_pool(name="const", bufs=1))
            persist = ctx.enter_context(tc.tile_pool(name="persist", bufs=1))
            xin = ctx.enter_context(tc.tile_pool(name="xin", bufs=1))
            work = ctx.enter_context(tc.tile_pool(name="work", bufs=2))
            expool = ctx.enter_context(tc.tile_pool(name="expool", bufs=4))
            npool = ctx.enter_context(tc.tile_pool(name="npool", bufs=3))
            psum = ctx.enter_context(tc.tile_pool(name="psum", bufs=2, space="PSUM"))

            ident = const.tile([P, P], BF16)
            make_identity(nc, ident)

            # Persistent SBUF: Qt/Kt [hd2, tokens], V-natural-with-ones, w_o.
            qt_sb = persist.tile([P, T], BF16, tag="qt_sb")
            kt_sb = persist.tile([P, T], BF16, tag="kt_sb")
            v_all = persist.tile([P, B * NKT * VROW], BF16, tag="v_all")
            wo_sb = persist.tile([P, NCH * 1024], BF16, tag="wo_sb")
            wq_sb = persist.tile([P, NCH * P], BF16, tag="wq_sb")
            wk_sb = persist.tile([P, NCH * P], BF16, tag="wk_sb")
            wv_sb = persist.tile([P, NCH * P], BF16, tag="wv_sb")

            # ones columns of v_all (cols 64 and 129 of each 130-block)
            v_view = v_all[:].rearrange("p (n c) -> p n c", c=VROW)
            nc.vector.memset(v_view[:, :, HD], 1.0)
            nc.vector.memset(v_view[:, :, 2 * HD + 1], 1.0)

            for ch in range(NCH):
                nc.sync.dma_start(wq_sb[:, ch * P:(ch + 1) * P], wq_d.ap()[ch])
                nc.sync.dma_start(wk_sb[:, ch * P:(ch + 1) * P], wk_d.ap()[ch])
                nc.sync.dma_start(wv_sb[:, ch * P:(ch + 1) * P], wv_d.ap()[ch])

            # ---- projection emitter (one 1024-token tile) ----
            def proj_unit(t2, sel):
                for which, x_d, w_sb in (("q", xq_d, wq_sb), ("k", xk_d, wk_sb),
                                         ("v", xv_d, wv_sb)):
                    if which != sel:
                        continue
                    xt = xin.tile([P, NCH * 1024], BF16, tag=f"x{which}")
                    for ch in range(NCH):
                        nc.sync.dma_start(xt[:, ch * 1024:(ch + 1) * 1024],
                                          x_d.ap()[t2, ch])
                    for half in range(2):
                        ps = psum.tile([P, 1024], F32, tag="sc")
                        for ch in range(NCH):
                            nc.tensor.matmul(
                                ps[:, 0:512], w_sb[:, ch * P:(ch + 1) * P],
                                xt[:, ch * 1024 + half * 512: ch * 1024 + (half + 1) * 512],
                                start=(ch == 0), stop=(ch == NCH - 1))
                        col = t2 * 1024 + half * 512
                        if which == "q":
                            nc.vector.tensor_copy(qt_sb[:, col:col + 512], ps[:, 0:512])
                        elif which == "k":
                            nc.vector.tensor_copy(kt_sb[:, col:col + 512], ps[:, 0:512])
                        else:
                            vt_scr = work.tile([P, 512], BF16, tag="vt_scr")
                            nc.vector.tensor_copy(vt_scr[:], ps[:, 0:512])
                            # transposes reuse the idle upper half of the same
                            # psum tile (bank 1) instead of new slot rotations
                            tp4 = ps[:, 512:768].bitcast(BF16)
                            for j in range(4):
                                tp = tp4[:, j * P:(j + 1) * P]
                                nc.tensor.transpose(
                                    tp[:], vt_scr[:, j * P:(j + 1) * P], ident[:])
                                g = col + j * P
                                b, kt = g // S, (g % S) // P
                                base = (b * NKT + kt) * VROW
                                # one strided copy: [V_h0|V_h1] -> cols
                                # [base:base+64] and [base+65:base+129]
                                nc.vector.tensor_copy(
                                    v_all[:, base:base + VROW]
                                    .rearrange("p (b c) -> p b c", c=HD + 1)[:, :, 0:HD],
                                    tp[:].rearrange("p (b c) -> p b c", c=HD))

            # ---- attention unit emitter ----
            def attn_unit(b, qw):
                        unit = b * NQW + qw
                        buf = qw % 2
                        dest = 2 * b + qw // 2
                        qcol = b * S + qw * 512
                        pv0 = psum.tile([HD + 1, 512], F32, tag="pv0")
                        pv1 = psum.tile([HD + 1, 512], F32, tag="pv1")
                        for kt in range(NKT):
                            kcol = b * S + kt * P
                            sc = psum.tile([P, 1024], F32, tag="sc")
                            nc.tensor.matmul(
                                sc[:, 0:512],
                                kt_sb[0:HD, kcol:kcol + P],
                                qt_sb[0:HD, qcol:qcol + 512],
                                start=True, stop=True, tile_position=(0, 0))
                            nc.tensor.matmul(
                                sc[:, 512:1024],
                                kt_sb[HD:2 * HD, kcol:kcol + P],
                                qt_sb[HD:2 * HD, qcol:qcol + 512],
                                start=True, stop=True, tile_position=(HD, 0))
                            ex = expool.tile([P, 1024], BF16, tag="ex")
                            nc.scalar.activation(
                                ex[:], sc[:], mybir.ActivationFunctionType.Exp)
                            vb = (b * NKT + kt) * VROW
                            nc.tensor.matmul(
                                pv0[:], v_all[:, vb:vb + HD + 1],
                                ex[:, 0:512],
                                start=(kt == 0), stop=(kt == NKT - 1))
                            nc.tensor.matmul(
                                pv1[:], v_all[:, vb + HD + 1:vb + VROW],
                                ex[:, 512:1024],
                                start=(kt == 0), stop=(kt == NKT - 1))
                        # normalize: rowsum rows -> DRAM -> [128,8] reciprocal
                        # -> DRAM -> partition-broadcast -> multiply.
                        rs = npool.tile([HD + 1, 1024], F32, tag="rs")
                        nc.vector.tensor_copy(rs[HD:HD + 1, 0:512], pv0[HD:HD + 1, :])
                        nc.vector.tensor_copy(rs[HD:HD + 1, 512:1024], pv1[HD:HD + 1, :])
                        nc.gpsimd.dma_start(
                            rs_d.ap()[unit].rearrange("(a f) -> a f", a=1),
                            rs[HD:HD + 1, :])
                        rsw = npool.tile([P, 8], F32, tag="rsw")
                        nc.gpsimd.dma_start(
                            rsw[:], rs_d.ap()[unit].rearrange("(p f) -> p f", f=8))
                        rcw = npool.tile([P, 8], F32, tag="rcw")
                        nc.vector.reciprocal(rcw[:], rsw[:])
                        nc.gpsimd.dma_start(
                            rcp_d.ap()[unit].rearrange("(p f) -> p f", f=8),
                            rcw[:])
                        bc0 = npool.tile([HD, 512], F32, tag="bc0")
                        bc1 = npool.tile([HD, 512], F32, tag="bc1")
                        nc.gpsimd.dma_start(
                            bc0[:], rcp_d.ap()[unit].rearrange("(a f) -> a f", a=1)[:, 0:512].to_broadcast((HD, 512)))
                        nc.gpsimd.dma_start(
                            bc1[:], rcp_d.ap()[unit].rearrange("(a f) -> a f", a=1)[:, 512:1024].to_broadcast((HD, 512)))
                        at0 = npool.tile([HD, 512], BF16, tag="at0")
                        at1 = npool.tile([HD, 512], BF16, tag="at1")
                        nc.vector.tensor_mul(at0[:], pv0[0:HD, :], bc0[:])
                        nc.vector.tensor_mul(at1[:], pv1[0:HD, :], bc1[:])
                        nc.gpsimd.dma_start(a2a_in[buf].ap()[dest][0:HD, :], at0[:])
                        nc.gpsimd.dma_start(a2a_in[buf].ap()[dest][HD:2 * HD, :], at1[:])

            def collective(buf):
                nc.gpsimd.collective_compute(
                    "AllToAll", mybir.AluOpType.bypass,
                    replica_groups=[list(range(NCORES))],
                    ins=[a2a_in[buf].ap()], outs=[a2a_out[buf].ap()],
                )

            # ---- output projection for one half of our token slice ----
            def outproj(buf):
                gsb = persist.tile([P, NCH * 512], BF16, tag=f"gsb{buf}")
                for ch in range(NCH):
                    nc.sync.dma_start(gsb[:, ch * 512:(ch + 1) * 512],
                                      a2a_out[buf].ap()[ch])
                for t128 in range(4):
                    for dhalf in range(2):
                        po = psum.tile([P, 512], F32,
                                       tag="pv0" if t128 % 2 == 0 else "pv1")
                        for ch in range(NCH):
                            nc.tensor.matmul(
                                po[:, 0:512],
                                gsb[:, ch * 512 + t128 * P: ch * 512 + (t128 + 1) * P],
                                wo_sb[:, ch * 1024 + dhalf * 512: ch * 1024 + (dhalf + 1) * 512],
                                start=(ch == 0), stop=(ch == NCH - 1))
                        osb = work.tile([P, 512], F32, tag="osb")
                        nc.vector.tensor_copy(osb[:], po[:])
                        row = buf * 512 + t128 * P
                        nc.sync.dma_start(
                            out_d.ap()[row:row + P, dhalf * 512:(dhalf + 1) * 512],
                            osb[:])

            for t2 in (0, 1):
                for sel in ("q", "k", "v"):
                    proj_unit(t2, sel)
            # w_o after the critical batch-0 inputs (needed only at outproj)
            for ch in range(NCH):
                nc.sync.dma_start(wo_sb[:, ch * 1024:(ch + 1) * 1024],
                                  wo_d.ap()[ch])
            for b in range(B):
                if b < B - 1:
                    pu = [(2 * b + 2 + i, sel) for i in (0, 1)
                          for sel in ("q", "k", "v")]
                    # interleave: attn unit, then 1-2 proj tensor-tiles
                    attn_unit(b, 0)
                    proj_unit(*pu[0]); proj_unit(*pu[1])
                    attn_unit(b, 2)
                    proj_unit(*pu[2]); proj_unit(*pu[3])
                    attn_unit(b, 1)
                    proj_unit(*pu[4]); proj_unit(*pu[5])
                    attn_unit(b, 3)
                else:
                    for qw in (0, 2):
                        attn_unit(b, qw)
            collective(0)
            for qw in (1, 3):
                attn_unit(B - 1, qw)
            collective(1)
            outproj(0)
            outproj(1)

    if split:
        split_multi_waits(nc)
        _CACHED_NC = nc
    return nc


def _host_prep(query, key, value, w_q, w_k, w_v, w_o):
    sc = 1.0 / np.sqrt(np.float32(HD))

    def tile_x(x):  # [B,S,D] -> [NT2, NCH, 128, 1024] bf16 of x^T
        xt = np.asarray(x, np.float32).reshape(T, D).T          # [D, T]
        xt = xt.reshape(NCH, P, NT2, 1024).transpose(2, 0, 1, 3)
        return np.ascontiguousarray(xt.astype(bf16))

    xq, xk, xv = tile_x(query), tile_x(key), tile_x(value)

    def tile_w(w, c, scale=1.0):  # rows for core c, transposed, chunked
        wc = (np.asarray(w, np.float32)[P * c:P * (c + 1), :] * scale).T  # [D,128]
        return np.ascontiguousarray(wc.reshape(NCH, P, P).astype(bf16))

    wo_t = np.ascontiguousarray(
        np.asarray(w_o, np.float32).T.reshape(NCH, P, 1024).astype(bf16))

    in_maps = []
    for c in range(NCORES):
        in_maps.append({
            "xq": xq, "xk": xk, "xv": xv,
            "wq": tile_w(w_q, c, sc), "wk": tile_w(w_k, c),
            "wv": tile_w(w_v, c), "wo": wo_t,
        })
    return in_maps


def _numpy_fallback(query, key, value, attn_mask, key_padding_mask,
                    w_q, b_q, w_k, b_k, w_v, b_v, w_o, b_o):
    q = query.reshape(T, D) @ w_q.T + b_q
    k = key.reshape(T, D) @ w_k.T + b_k
    v = value.reshape(T, D) @ w_v.T + b_v
    qh = q.reshape(B, S, H, HD).transpose(0, 2, 1, 3)
    kh = k.reshape(B, S, H, HD).transpose(0, 2, 1, 3)
    vh = v.reshape(B, S, H, HD).transpose(0, 2, 1, 3)
    out = np.empty((B, H, S, HD), np.float32)
    neg = np.finfo(np.float32).min
    for b in range(B):
        for h in range(H):
            s = (qh[b, h] @ kh[b, h].T) / np.sqrt(np.float32(HD))
            s = np.where(attn_mask, s, neg)
            s = np.where(key_padding_mask[b][None, :], s, neg)
            s = s - s.max(axis=-1, keepdims=True)
            e = np.exp(s)
            a = e / e.sum(axis=-1, keepdims=True)
            out[b, h] = a @ vh[b, h]
    o = out.transpose(0, 2, 1, 3).reshape(T, D)
    return (o @ w_o.T + b_o).reshape(B, S, D).astype(np.float32)


def kernel(query, key, value, attn_mask, key_padding_mask,
           w_q, b_q, w_k, b_k, w_v, b_v, w_o, b_o):
    query = np.asarray(query, np.float32)
    key = np.asarray(key, np.float32)
    value = np.asarray(value, np.float32)
    attn_mask = np.asarray(attn_mask)
    key_padding_mask = np.asarray(key_padding_mask)
    w_q, b_q = np.asarray(w_q, np.float32), np.asarray(b_q, np.float32)
    w_k, b_k = np.asarray(w_k, np.float32), np.asarray(b_k, np.float32)
    w_v, b_v = np.asarray(w_v, np.float32), np.asarray(b_v, np.float32)
    w_o, b_o = np.asarray(w_o, np.float32), np.asarray(b_o, np.float32)

    if (not attn_mask.all() or not key_padding_mask.all()
            or b_q.any() or b_k.any()):
        return _numpy_fallback(query, key, value, attn_mask, key_padding_mask,
                               w_q, b_q, w_k, b_k, w_v, b_v, w_o, b_o)

    nc = build()
    in_maps = _host_prep(query, key, value, w_q, w_k, w_v, w_o)
    res = run_bass_kernel_spmd(nc, in_maps, list(range(NCORES)))

    out = np.empty((T, D), np.float32)
    for c in range(NCORES):
        out[TOK_PER_CORE * c:TOK_PER_CORE * (c + 1)] = \
            res.results[c]["out"].reshape(TOK_PER_CORE, D)
    # exact host-side bias fold: softmax rows sum to 1 => + (b_v @ w_o.T + b_o)
    out += b_v @ w_o.T + b_o
    return out.reshape(B, S, D)
